# revision 1
# baseline (speedup 1.0000x reference)
"""Dense transformer block (rmsnorm+causal attention+rope / rmsnorm+SwiGLU) on 8 TRN2 cores.

Sharding:
  core j (j=0..7): batch b = j//4, head-group hg = j%4 (heads 4*hg..4*hg+3).
  Phase A (attention) is head-sharded: each core computes rmsnorm(x[b]) -> QKV for
  its 4 heads -> rope -> causal attention -> O^T [512, T].
  Two 8-core AllToAlls (heads {0,1} then {2,3}) reshard to row-sharding; the first
  overlaps the second half of attention, the second overlaps the first half of the
  output projection. Cross-batch slices are neutralized by zero rows in a
  per-core-padded w_proj.
  Phase B (proj residual + rmsnorm2 + SwiGLU MLP) is row-sharded: each core
  computes its 512 rows against full weights; outputs are concatenated on host.

Matmul operands are bf16 (weights pre-cast on host, w_norm folded into weight
rows); statistics, softmax denominators, residual stream and PSUM stay fp32.
"""

import numpy as np
import ml_dtypes

import concourse.bass as bass
import concourse.mybir as mybir
import concourse.tile as tile
from concourse import bacc
from concourse import bass_utils
from concourse.masks import make_identity

AF = mybir.ActivationFunctionType
ALU = mybir.AluOpType
F32 = mybir.dt.float32
BF16 = mybir.dt.bfloat16
MMDT = BF16
NP_MMDT = ml_dtypes.bfloat16

P = 128
T = 2048
C = 2048
D = 128
H = 16
HPC = 4          # heads per core
HID = 5632
HID_T = HID // P  # 44 hid tiles
TQ = 512         # q-chunk / output row-block per core
EPS = 1e-6
ROPE_BASE = 10000.0
CT = C // P      # 16 contraction tiles
QKV_CHUNK = 1024


def _build():
    nc = bacc.Bacc(None, target_bir_lowering=False, num_devices=8)

    # ---- kernel I/O ----
    x_full = nc.dram_tensor("x_full", [T, C], F32, kind="ExternalInput")
    x_t = nc.dram_tensor("x_t", [C, T], F32, kind="ExternalInput")
    x_rows = nc.dram_tensor("x_rows", [TQ, C], F32, kind="ExternalInput")
    wq = nc.dram_tensor("wq", [C, HPC * D], MMDT, kind="ExternalInput")
    wk = nc.dram_tensor("wk", [C, HPC * D], MMDT, kind="ExternalInput")
    wv = nc.dram_tensor("wv", [C, HPC * D], MMDT, kind="ExternalInput")
    wpe = nc.dram_tensor("wpe", [2 * C, C], MMDT, kind="ExternalInput")
    w1t = nc.dram_tensor("w1t", [HID_T, C, P], MMDT, kind="ExternalInput")
    w2t = nc.dram_tensor("w2t", [HID_T, C, P], MMDT, kind="ExternalInput")
    w3 = nc.dram_tensor("w3", [HID, C], MMDT, kind="ExternalInput")
    rope_t = nc.dram_tensor("rope_t", [D, T], F32, kind="ExternalInput")
    tri = nc.dram_tensor("tri", [P, P], MMDT, kind="ExternalInput")
    out = nc.dram_tensor("out", [TQ, C], F32, kind="ExternalOutput")

    inv_sqrt_d = 1.0 / float(np.sqrt(D))

    with tile.TileContext(nc) as tc:
        with (
            tc.tile_pool(name="const", bufs=1) as const,
            tc.tile_pool(name="dram", bufs=1, space="DRAM") as dram,
        ):
            # ---- constants ----
            ident_f = const.tile([P, P], F32)
            make_identity(nc, ident_f)
            ident = const.tile([P, P], MMDT)
            nc.vector.tensor_copy(out=ident, in_=ident_f)
            ones_f = const.tile([P, 1], F32)
            nc.vector.memset(ones_f, 1.0)
            ones_r = const.tile([P, 1], MMDT)
            nc.vector.tensor_copy(out=ones_r, in_=ones_f)
            eps_sb = const.tile([P, 1], F32)
            nc.vector.memset(eps_sb, EPS)
            rope_sb = const.tile([D, T], F32)
            nc.sync.dma_start(out=rope_sb, in_=rope_t[:, :])
            tri_sb = const.tile([P, P], MMDT)
            nc.sync.dma_start(out=tri_sb, in_=tri[:, :])
            rstd_T = const.tile([1, T], F32)

            # ---- DRAM scratch ----
            qT_d = dram.tile([HPC * D, T], MMDT)
            kT_d = dram.tile([HPC * D, T], MMDT)
            v_d = dram.tile([T, HPC * D], MMDT)
            xmid_d = dram.tile([TQ, C], F32)
            a2a1_in = dram.tile([8, 2 * P, TQ], MMDT)
            a2a1_out = dram.tile([8, 2 * P, TQ], MMDT)
            a2a2_in = dram.tile([8, 2 * P, TQ], MMDT)
            a2a2_out = dram.tile([8, 2 * P, TQ], MMDT)

            # ================= Phase A1+A2: rmsnorm1 + h^T + QKV =================
            with (
                tc.tile_pool(name="p12", bufs=2) as p12,
                tc.tile_pool(name="p12psum", bufs=2, space="PSUM") as pp12,
            ):
                CHUNKS = [256, 256, 512, 1024]
                t0 = 0
                for ch, CHW in enumerate(CHUNKS):
                    rt_per_chunk = CHW // P
                    SUBW = min(TQ, CHW)
                    # rstd for this chunk's rows -> rstd_T[0, t0:t0+chunk] (via PE transpose)
                    for rt in range(rt_per_chunk):
                        row0 = t0 + rt * P
                        xt = p12.tile([P, C], F32, tag="xt", bufs=2)
                        nc.sync.dma_start(out=xt, in_=x_full[row0 : row0 + P, :])
                        sq = p12.tile([P, C], F32, tag="sq", bufs=1)
                        ssum = p12.tile([P, 1], F32, tag="ssum", bufs=3)
                        nc.scalar.activation(sq, xt, AF.Square, accum_out=ssum)
                        rstd = p12.tile([P, 1], F32, tag="rstd", bufs=3)
                        nc.scalar.activation(rstd, ssum, AF.Sqrt, bias=eps_sb, scale=1.0 / C)
                        nc.vector.reciprocal(out=rstd, in_=rstd)
                        prs = pp12.tile([1, P], F32, tag="rsT", bufs=2)
                        nc.tensor.matmul(prs, rstd, ident_f, start=True, stop=True)
                        nc.scalar.activation(rstd_T[0:1, row0 : row0 + P], prs, AF.Copy)

                    rstd_bc = p12.tile([P, QKV_CHUNK], F32, tag="rstd_bc", bufs=2, name="rstd_bc")[
                        :, :CHW
                    ]
                    nc.gpsimd.partition_broadcast(
                        rstd_bc[:], rstd_T[0:1, t0 : t0 + CHW]
                    )
                    hT = p12.tile([P, CT, QKV_CHUNK], MMDT, tag="hT", bufs=2, name="hT")[:, :, :CHW]
                    for ct in range(CT):
                        xtt = p12.tile([P, QKV_CHUNK], F32, tag="xtt", bufs=3, name="xtt")[:, :CHW]
                        nc.sync.dma_start(
                            out=xtt, in_=x_t[ct * P : (ct + 1) * P, t0 : t0 + CHW]
                        )
                        nc.vector.tensor_tensor(
                            out=hT[:, ct, :], in0=xtt, in1=rstd_bc, op=ALU.mult
                        )

                    # q^T / k^T with fused rope on eviction
                    for which, w_in, dst in (("q", wq, qT_d), ("k", wk, kT_d)):
                        for m in range(HPC):
                            wt = p12.tile([P, CT, P], MMDT, tag="wt", bufs=3)
                            nc.sync.dma_start(
                                out=wt,
                                in_=w_in[:, m * P : (m + 1) * P].rearrange(
                                    "(ct p) d -> p ct d", p=P
                                ),
                            )
                            for sub in range(CHW // SUBW):
                                s0 = sub * SUBW
                                g0 = t0 + s0
                                pq = pp12.tile([P, TQ], F32, tag="qk", bufs=3, name="pq")[:, :SUBW]
                                for ct in range(CT):
                                    nc.tensor.matmul(
                                        pq,
                                        wt[:, ct, :],
                                        hT[:, ct, s0 : s0 + SUBW],
                                        start=(ct == 0),
                                        stop=(ct == CT - 1),
                                    )
                                # rope: rows 0:64 = x1*cos - x2*sin ; 64:128 = x1*sin + x2*cos
                                HD2 = D // 2
                                x1 = pq[0:HD2, :]
                                x2 = pq[HD2:P, :]
                                cosw = rope_sb[0:HD2, g0 : g0 + SUBW]
                                sinw = rope_sb[HD2:D, g0 : g0 + SUBW]
                                rop = p12.tile([P, TQ], MMDT, tag="rope_out", bufs=4, name="rop")[
                                    :, :SUBW
                                ]
                                tm1 = p12.tile([HD2, TQ], F32, tag="tm1", bufs=2, name="tm1")[:, :SUBW]
                                tm2 = p12.tile([HD2, TQ], F32, tag="tm2", bufs=2, name="tm2")[:, :SUBW]
                                nc.vector.tensor_tensor(out=tm1, in0=x1, in1=cosw, op=ALU.mult)
                                nc.vector.tensor_tensor(out=tm2, in0=x2, in1=sinw, op=ALU.mult)
                                nc.vector.tensor_tensor(
                                    out=rop[0:HD2, :], in0=tm1, in1=tm2, op=ALU.subtract
                                )
                                nc.vector.tensor_tensor(out=tm1, in0=x1, in1=sinw, op=ALU.mult)
                                nc.vector.tensor_tensor(out=tm2, in0=x2, in1=cosw, op=ALU.mult)
                                nc.vector.tensor_tensor(
                                    out=rop[HD2:P, :], in0=tm1, in1=tm2, op=ALU.add
                                )
                                nc.sync.dma_start(
                                    out=dst[m * P : (m + 1) * P, g0 : g0 + SUBW], in_=rop
                                )

                    # v in row layout [T, HPC*D]
                    wv_sb = p12.tile([P, CT, HPC * D], MMDT, tag="wv_sb", bufs=1)
                    nc.sync.dma_start(
                        out=wv_sb, in_=wv.rearrange("(ct p) d -> p ct d", p=P)
                    )
                    for rt in range(rt_per_chunk):
                        pv = pp12.tile([P, HPC * D], F32, tag="v", bufs=3)
                        for ct in range(CT):
                            nc.tensor.matmul(
                                pv,
                                hT[:, ct, rt * P : (rt + 1) * P],
                                wv_sb[:, ct, :],
                                start=(ct == 0),
                                stop=(ct == CT - 1),
                            )
                        vt = p12.tile([P, HPC * D], MMDT, tag="vt", bufs=3)
                        nc.scalar.activation(vt, pv, AF.Copy)
                        nc.sync.dma_start(
                            out=v_d[t0 + rt * P : t0 + (rt + 1) * P, :], in_=vt
                        )
                    t0 += CHW

            # ================= Phase A3: causal attention (+ split A2A) ==========
            wpre_ctx = tc.tile_pool(name="wpre", bufs=1)
            wpre = wpre_ctx.__enter__()
            wpe_pre = wpre.tile([P, 2, 16, TQ], MMDT, tag="wpe_pre", bufs=1)
            lp0 = wpre.tile([P, 8, 2, TQ], MMDT, tag="lp0", bufs=1)
            lp1 = wpre.tile([P, 8, 2, TQ], MMDT, tag="lp1", bufs=1)
            with (
                tc.tile_pool(name="att", bufs=2) as att,
                tc.tile_pool(name="attpsum", bufs=2, space="PSUM") as pat,
            ):
                wpe_pre_ctr = [0]

                def _drip_wpe(n):
                    while wpe_pre_ctr[0] < 32 and n > 0:
                        i = wpe_pre_ctr[0]
                        cc_, sa_ = i // 16, i % 16
                        nc.sync.dma_start(
                            out=wpe_pre[:, cc_, sa_, :],
                            in_=wpe[sa_ * P : (sa_ + 1) * P, cc_ * TQ : (cc_ + 1) * TQ],
                        )
                        wpe_pre_ctr[0] += 1
                        n -= 1

                for h in range(HPC):
                    a2a_in = a2a1_in if h < 2 else a2a2_in
                    hrow0 = (h % 2) * P
                    kT_h = att.tile([P, T], MMDT, tag="kT_h", bufs=2)
                    nc.sync.dma_start(out=kT_h, in_=kT_d[h * P : (h + 1) * P, :])
                    v_h = att.tile([P, T // P, D], MMDT, tag="v_h", bufs=2)
                    nc.sync.dma_start(
                        out=v_h,
                        in_=v_d[:, h * P : (h + 1) * P].rearrange(
                            "(kb p) d -> p kb d", p=P
                        ),
                    )
                    for qc in range(T // TQ):
                        if h == 3 and qc == 3:
                            for s_ in range(8):
                                for a_ in range(2):
                                    nc.sync.dma_start(
                                        out=lp0[:, s_, a_, :],
                                        in_=a2a1_out[s_, a_ * P : (a_ + 1) * P, :],
                                    )
                        qT_c = att.tile([P, TQ], MMDT, tag="qT_c", bufs=3)
                        nc.sync.dma_start(
                            out=qT_c,
                            in_=qT_d[h * P : (h + 1) * P, qc * TQ : (qc + 1) * TQ],
                        )
                        nkb = 4 * qc + 4
                        l_ps = pat.tile([1, TQ], F32, tag="l", bufs=2)
                        o_ps = pat.tile([P, TQ], F32, tag="o", bufs=2)
                        es = []
                        for kb in range(nkb):
                            r = kb - 4 * qc
                            q0 = max(0, r * P)
                            st = pat.tile([P, TQ], F32, tag="st", bufs=4)
                            nc.tensor.matmul(
                                st[:, q0:TQ],
                                kT_h[:, kb * P : (kb + 1) * P],
                                qT_c[:, q0:TQ],
                                start=True,
                                stop=True,
                            )
                            e = att.tile([P, TQ], MMDT, tag="e", bufs=18)
                            nc.scalar.activation(
                                e[:, q0:TQ], st[:, q0:TQ], AF.Exp, scale=inv_sqrt_d
                            )
                            if r >= 0:
                                nc.vector.tensor_tensor(
                                    out=e[:, q0 : q0 + P],
                                    in0=e[:, q0 : q0 + P],
                                    in1=tri_sb,
                                    op=ALU.mult,
                                )
                            es.append((e, q0))
                        for kb in range(nkb):
                            e, q0 = es[kb]
                            nc.tensor.matmul(
                                l_ps[:, q0:TQ],
                                ones_r,
                                e[:, q0:TQ],
                                start=(kb == 0),
                                stop=(kb == nkb - 1),
                            )
                            nc.tensor.matmul(
                                o_ps[:, q0:TQ],
                                v_h[:, kb, :],
                                e[:, q0:TQ],
                                start=(kb == 0),
                                stop=(kb == nkb - 1),
                            )
                        l_inv = att.tile([1, TQ], F32, tag="l_inv", bufs=2)
                        nc.vector.reciprocal(out=l_inv, in_=l_ps)
                        l_bc = att.tile([P, TQ], F32, tag="l_bc", bufs=2)
                        nc.gpsimd.partition_broadcast(l_bc[:], l_inv[:])
                        oT = att.tile([P, TQ], MMDT, tag="oT", bufs=3)
                        nc.vector.tensor_tensor(out=oT, in0=o_ps, in1=l_bc, op=ALU.mult)
                        nc.sync.dma_start(out=a2a_in[qc, hrow0 : hrow0 + P, :], in_=oT)
                        nc.sync.dma_start(
                            out=a2a_in[qc + 4, hrow0 : hrow0 + P, :], in_=oT
                        )
                        _drip_wpe(2)
                    if h == 1:
                        nc.gpsimd.collective_compute(
                            "AllToAll",
                            ALU.bypass,
                            replica_groups=[[0, 1, 2, 3, 4, 5, 6, 7]],
                            ins=[a2a1_in.opt()],
                            outs=[a2a1_out.opt()],
                        )
                    if h == 3:
                        nc.gpsimd.collective_compute(
                            "AllToAll",
                            ALU.bypass,
                            replica_groups=[[0, 1, 2, 3, 4, 5, 6, 7]],
                            ins=[a2a2_in.opt()],
                            outs=[a2a2_out.opt()],
                        )

            # ================= Phase B1: proj + residual (two-stage) =============
            with (
                tc.tile_pool(name="proj", bufs=2) as prj,
                tc.tile_pool(name="projpsum", bufs=2, space="PSUM") as ppj,
            ):
                xr = prj.tile([P, 4, C], F32, tag="xr", bufs=1)
                nc.sync.dma_start(
                    out=xr, in_=x_rows.rearrange("(qt p) c -> p qt c", p=P)
                )
                y0acc = prj.tile([P, 4, C], F32, tag="y0acc", bufs=1)
                for half, a2a_o in ((0, a2a1_out), (1, a2a2_out)):
                    lp = lp0 if half == 0 else lp1
                    if half == 1:
                        for s_ in range(8):
                            for a_ in range(2):
                                nc.sync.dma_start(
                                    out=lp[:, s_, a_, :],
                                    in_=a2a_o[s_, a_ * P : (a_ + 1) * P, :],
                                )
                    for cc in range(4):
                        yps = [
                            ppj.tile([P, TQ], F32, tag=f"y{qt}", bufs=1, name=f"y{qt}")
                            for qt in range(4)
                        ]
                        for s in range(8):
                            for a in range(2):
                                if half == 0 and cc < 2:
                                    wt = wpe_pre[:, cc, s * 2 + a, :]
                                else:
                                    wt = prj.tile([P, TQ], MMDT, tag="wpe_t", bufs=6)
                                    nc.sync.dma_start(
                                        out=wt,
                                        in_=wpe[
                                            (half * 16 + s * 2 + a) * P
                                            : (half * 16 + s * 2 + a + 1) * P,
                                            cc * TQ : (cc + 1) * TQ,
                                        ],
                                    )
                                for qt in range(4):
                                    nc.tensor.matmul(
                                        yps[qt],
                                        lp[:, s, a, qt * P : (qt + 1) * P],
                                        wt,
                                        start=(s == 0 and a == 0),
                                        stop=(s == 7 and a == 1),
                                    )
                        for qt in range(4):
                            if half == 0:
                                nc.scalar.activation(
                                    y0acc[:, qt, cc * TQ : (cc + 1) * TQ],
                                    yps[qt],
                                    AF.Copy,
                                )
                            else:
                                t1 = prj.tile([P, TQ], F32, tag="t1", bufs=3)
                                nc.vector.tensor_tensor(
                                    out=t1,
                                    in0=yps[qt],
                                    in1=y0acc[:, qt, cc * TQ : (cc + 1) * TQ],
                                    op=ALU.add,
                                )
                                xm = prj.tile([P, TQ], F32, tag="xm", bufs=3)
                                nc.vector.tensor_tensor(
                                    out=xm,
                                    in0=t1,
                                    in1=xr[:, qt, cc * TQ : (cc + 1) * TQ],
                                    op=ALU.add,
                                )
                                nc.sync.dma_start(
                                    out=xmid_d[
                                        qt * P : (qt + 1) * P, cc * TQ : (cc + 1) * TQ
                                    ],
                                    in_=xm,
                                )

            wpre_ctx.__exit__(None, None, None)

            # ================= Phase B2: rmsnorm2 + h2^T + SwiGLU ================
            with tc.tile_pool(name="mlp", bufs=2) as mlp:
                h2T = mlp.tile([P, CT, TQ], MMDT, tag="h2T", bufs=1)
                with tc.tile_pool(name="pml_tr", bufs=2, space="PSUM") as pml_tr:
                    for rt in range(TQ // P):
                        xt = mlp.tile([P, C], F32, tag="xt2", bufs=2)
                        nc.sync.dma_start(out=xt, in_=xmid_d[rt * P : (rt + 1) * P, :])
                        sq = mlp.tile([P, C], F32, tag="sq2", bufs=1)
                        ssum = mlp.tile([P, 1], F32, tag="ssum2", bufs=2)
                        nc.scalar.activation(sq, xt, AF.Square, accum_out=ssum)
                        rstd = mlp.tile([P, 1], F32, tag="rstd2", bufs=2)
                        nc.scalar.activation(rstd, ssum, AF.Sqrt, bias=eps_sb, scale=1.0 / C)
                        nc.vector.reciprocal(out=rstd, in_=rstd)
                        hrow = mlp.tile([P, C], MMDT, tag="hrow2", bufs=2)
                        nc.vector.tensor_scalar(
                            out=hrow, in0=xt, scalar1=rstd, scalar2=None, op0=ALU.mult
                        )
                        for ct in range(CT):
                            ptr = pml_tr.tile([P, P], MMDT, tag="tr2", bufs=4)
                            nc.tensor.transpose(ptr, hrow[:, ct * P : (ct + 1) * P], ident)
                            nc.scalar.activation(
                                h2T[:, ct, rt * P : (rt + 1) * P], ptr, AF.Copy
                            )

                # ---- SwiGLU ----
                pml = ctx_pml = tc.tile_pool(name="pml_mm", bufs=2, space="PSUM")
                pml = pml.__enter__()
                HHALF = HID_T // 4  # 11
                NPART = 4
                y3acc = mlp.tile([P, 4, C], F32, tag="y3acc", bufs=1)
                for half in range(NPART):
                    uT = mlp.tile([P, HHALF, TQ], MMDT, tag="uT", bufs=2)
                    for ht in range(HHALF):
                        htg = half * HHALF + ht
                        w1_sb = mlp.tile([P, CT, P], MMDT, tag="w1_sb", bufs=3)
                        nc.sync.dma_start(
                            out=w1_sb,
                            in_=w1t[htg].rearrange("(ct p) d -> p ct d", p=P),
                        )
                        w2_sb = mlp.tile([P, CT, P], MMDT, tag="w2_sb", bufs=3)
                        nc.sync.dma_start(
                            out=w2_sb,
                            in_=w2t[htg].rearrange("(ct p) d -> p ct d", p=P),
                        )
                        g1 = pml.tile([P, TQ], F32, tag="g1", bufs=2)
                        g2 = pml.tile([P, TQ], F32, tag="g2", bufs=2)
                        for ct in range(CT):
                            nc.tensor.matmul(
                                g1,
                                w1_sb[:, ct, :],
                                h2T[:, ct, :],
                                start=(ct == 0),
                                stop=(ct == CT - 1),
                            )
                        for ct in range(CT):
                            nc.tensor.matmul(
                                g2,
                                w2_sb[:, ct, :],
                                h2T[:, ct, :],
                                start=(ct == 0),
                                stop=(ct == CT - 1),
                            )
                        sil = mlp.tile([P, TQ], F32, tag="sil", bufs=3)
                        nc.scalar.activation(sil, g1, AF.Silu)
                        nc.vector.tensor_tensor(
                            out=uT[:, ht, :], in0=g2, in1=sil, op=ALU.mult
                        )
                    for cc in range(4):
                        y3ps = [
                            pml.tile(
                                [P, TQ], F32, tag=f"y3_{rt}", bufs=1, name=f"y3_{rt}"
                            )
                            for rt in range(4)
                        ]
                        for ht in range(HHALF):
                            htg = half * HHALF + ht
                            w3_sb = mlp.tile([P, TQ], MMDT, tag="w3_sb", bufs=4)
                            nc.sync.dma_start(
                                out=w3_sb,
                                in_=w3[htg * P : (htg + 1) * P, cc * TQ : (cc + 1) * TQ],
                            )
                            for rt in range(4):
                                nc.tensor.matmul(
                                    y3ps[rt],
                                    uT[:, ht, rt * P : (rt + 1) * P],
                                    w3_sb,
                                    start=(ht == 0),
                                    stop=(ht == HHALF - 1),
                                )
                        for rt in range(4):
                            if half == 0:
                                nc.scalar.activation(
                                    y3acc[:, rt, cc * TQ : (cc + 1) * TQ],
                                    y3ps[rt],
                                    AF.Copy,
                                )
                            elif half < NPART - 1:
                                nc.vector.tensor_tensor(
                                    out=y3acc[:, rt, cc * TQ : (cc + 1) * TQ],
                                    in0=y3ps[rt],
                                    in1=y3acc[:, rt, cc * TQ : (cc + 1) * TQ],
                                    op=ALU.add,
                                )
                            else:
                                xmt = mlp.tile([P, TQ], F32, tag="xmt", bufs=3)
                                nc.sync.dma_start(
                                    out=xmt,
                                    in_=xmid_d[
                                        rt * P : (rt + 1) * P, cc * TQ : (cc + 1) * TQ
                                    ],
                                )
                                osum = mlp.tile([P, TQ], F32, tag="osum", bufs=3)
                                nc.vector.tensor_tensor(
                                    out=osum,
                                    in0=y3ps[rt],
                                    in1=y3acc[:, rt, cc * TQ : (cc + 1) * TQ],
                                    op=ALU.add,
                                )
                                ofin = mlp.tile([P, TQ], F32, tag="ofin", bufs=3)
                                nc.vector.tensor_tensor(
                                    out=ofin, in0=osum, in1=xmt, op=ALU.add
                                )
                                nc.sync.dma_start(
                                    out=out[
                                        rt * P : (rt + 1) * P, cc * TQ : (cc + 1) * TQ
                                    ],
                                    in_=ofin,
                                )
                ctx_pml.__exit__(None, None, None)

    nc.compile()
    return nc


_NC_CACHE = None


def _get_nc():
    global _NC_CACHE
    if _NC_CACHE is None:
        _NC_CACHE = _build()
    return _NC_CACHE


def _host_inputs(x, w_norm1, w_qkv, w_proj, w_norm2, w1, w2, w3):
    x = np.asarray(x, dtype=np.float32)
    w_qkv = np.asarray(w_qkv, dtype=np.float32)
    w_proj = np.asarray(w_proj, dtype=np.float32)
    w_norm1 = np.asarray(w_norm1, dtype=np.float32)
    w_norm2 = np.asarray(w_norm2, dtype=np.float32)
    w1 = np.asarray(w1, dtype=np.float32)
    w2 = np.asarray(w2, dtype=np.float32)
    w3 = np.asarray(w3, dtype=np.float32)

    half = D // 2
    inv_freq = 1.0 / (ROPE_BASE ** (np.arange(half, dtype=np.float32) / half))
    pos = np.arange(T, dtype=np.float32)
    freqs = pos[:, None] * inv_freq[None, :]
    rope_tab = np.ascontiguousarray(
        np.concatenate([np.cos(freqs).T, np.sin(freqs).T], axis=0).astype(np.float32)
    )

    ql = np.arange(P)[None, :]
    kv = np.arange(P)[:, None]
    tri = (ql >= kv).astype(NP_MMDT)

    # fold w_norm into weight rows (h @ W == (x*rstd) @ (diag(wn) W))
    w_qkv_n = w_qkv * w_norm1[:, None]
    w1_n = w1 * w_norm2[:, None]
    w2_n = w2 * w_norm2[:, None]

    w1t = np.ascontiguousarray(
        w1_n.reshape(C, HID_T, P).transpose(1, 0, 2)
    ).astype(NP_MMDT)
    w2t = np.ascontiguousarray(
        w2_n.reshape(C, HID_T, P).transpose(1, 0, 2)
    ).astype(NP_MMDT)
    w3_b = w3.astype(NP_MMDT)

    in_maps = []
    for j in range(8):
        b, hg = j // 4, j % 4
        col0 = hg * HPC * D
        # wpe rows: [half(2), sender p(8), a(2), d(128)]; sender p's head = 4*(p%4)+half*2+a
        wpe = np.zeros((2 * C, C), dtype=NP_MMDT)
        for hf in range(2):
            for p_ in range(8):
                if p_ // 4 != b:
                    continue
                for a in range(2):
                    gh = 4 * (p_ % 4) + hf * 2 + a
                    dst = (hf * 16 + p_ * 2 + a) * P
                    wpe[dst : dst + P, :] = w_proj[gh * P : (gh + 1) * P, :].astype(
                        NP_MMDT
                    )
        xb = np.ascontiguousarray(x[b])
        in_maps.append(
            {
                "x_full": xb,
                "x_t": np.ascontiguousarray(xb.T),
                "x_rows": np.ascontiguousarray(xb[hg * TQ : (hg + 1) * TQ]),
                "wq": np.ascontiguousarray(
                    w_qkv_n[:, col0 : col0 + HPC * D]
                ).astype(NP_MMDT),
                "wk": np.ascontiguousarray(
                    w_qkv_n[:, C + col0 : C + col0 + HPC * D]
                ).astype(NP_MMDT),
                "wv": np.ascontiguousarray(
                    w_qkv_n[:, 2 * C + col0 : 2 * C + col0 + HPC * D]
                ).astype(NP_MMDT),
                "wpe": wpe,
                "w1t": w1t,
                "w2t": w2t,
                "w3": w3_b,
                "rope_t": rope_tab,
                "tri": tri,
            }
        )
    return in_maps


def kernel(x, w_norm1, w_qkv, w_proj, w_norm2, w1, w2, w3, _trace=False, _tmpdir=None):
    nc = _get_nc()
    in_maps = _host_inputs(x, w_norm1, w_qkv, w_proj, w_norm2, w1, w2, w3)
    kwargs = {}
    if _trace:
        kwargs = {"trace": True, "tmpdir": _tmpdir}
    res = bass_utils.run_bass_kernel_spmd(
        nc, in_maps, core_ids=list(range(8)), **kwargs
    )
    out = np.empty((2, T, C), dtype=np.float32)
    for j in range(8):
        out[j // 4, (j % 4) * TQ : (j % 4 + 1) * TQ, :] = res.results[j]["out"]
    kernel._last_exec_time_ns = res.exec_time_ns
    return out



# revision 29
# speedup vs baseline: 1.1750x; 1.1750x over previous
"""Dense transformer block (rmsnorm+causal attention+rope / rmsnorm+SwiGLU) on 8 TRN2 cores.

Sharding:
  core j (j=0..7): batch b = j//4, head-group hg = j%4 (heads 4*hg..4*hg+3).
  Phase A (attention) is head-sharded: each core computes rmsnorm(x[b]) -> QKV for
  its 4 heads -> rope -> causal attention -> O^T [128, T] per head, with q/k/v
  kept SBUF-resident.  rmsnorm stats come from a ones-matmul over x^2 in
  transposed layout (no row-major x pass needed).
  Two 4-core AllToAlls (heads {0,1} then {2,3}, grouped by batch) reshard to
  query-sharding; the first overlaps attention heads 2-3, the second overlaps
  the first half of the output projection.
  Phase B runs fully transposed: proj y^T accumulates [C, TQ] directly
  (stationary w_proj blocks, moving attention outputs), rmsnorm2 stats via
  ones-matmul, SwiGLU with a transposed w3 pass; the kernel output is [C, TQ]
  per core and is transposed on host.

Matmul operands are bf16 (weights and x pre-cast on host, w_norm folded into
weight rows); statistics, softmax denominators, residual stream and PSUM stay
fp32 (residual x arrives separately as fp32 x_tm).
"""

import numpy as np
import ml_dtypes

import concourse.bass as bass
import concourse.mybir as mybir
import concourse.tile as tile
from concourse import bacc
from concourse import bass_utils

AF = mybir.ActivationFunctionType
ALU = mybir.AluOpType
F32 = mybir.dt.float32
BF16 = mybir.dt.bfloat16
MMDT = BF16
NP_MMDT = ml_dtypes.bfloat16

P = 128
T = 2048
C = 2048
D = 128
H = 16
HPC = 4          # heads per core
HID = 5632
HID_T = HID // P  # 44 hid tiles
TQ = 512         # q-chunk / output col-block per core
EPS = 1e-6
ROPE_BASE = 10000.0
CT = C // P      # 16 contraction tiles
NCH = 4          # QKV t-chunks of 512


def _build():
    nc = bacc.Bacc(None, target_bir_lowering=False, num_devices=8)

    # ---- kernel I/O ----
    x_t = nc.dram_tensor("x_t", [C, T], MMDT, kind="ExternalInput")
    x_tm = nc.dram_tensor("x_tm", [C, TQ], F32, kind="ExternalInput")
    wq = nc.dram_tensor("wq", [P, CT, HPC * D], MMDT, kind="ExternalInput")
    wk = nc.dram_tensor("wk", [P, CT, HPC * D], MMDT, kind="ExternalInput")
    wv = nc.dram_tensor("wv", [P, CT, HPC * D], MMDT, kind="ExternalInput")
    wpe_r = nc.dram_tensor("wpe_r", [2, CT, P, 8 * P], MMDT, kind="ExternalInput")
    bmask = nc.dram_tensor("bmask", [P, 2], F32, kind="ExternalInput")
    w1t = nc.dram_tensor("w1t", [HID_T, P, CT * P], MMDT, kind="ExternalInput")
    w2t = nc.dram_tensor("w2t", [HID_T, P, CT * P], MMDT, kind="ExternalInput")
    w3r = nc.dram_tensor("w3r", [CT, P, HID_T * P], MMDT, kind="ExternalInput")
    rope_t = nc.dram_tensor("rope_t", [D, T], F32, kind="ExternalInput")
    tri = nc.dram_tensor("tri", [P, P], MMDT, kind="ExternalInput")
    out = nc.dram_tensor("out", [C, TQ], F32, kind="ExternalOutput")

    inv_sqrt_d = 1.0 / float(np.sqrt(D))
    GROUPS = [[0, 1, 2, 3, 4, 5, 6, 7]]

    with tile.TileContext(nc) as tc:
        with (
            tc.tile_pool(name="const", bufs=1) as const,
            tc.tile_pool(name="dram", bufs=1, space="DRAM") as dram,
        ):
            # ---- constants ----
            ones_f = const.tile([P, 1], F32)
            nc.vector.memset(ones_f, 1.0)
            ones_r = const.tile([P, 1], MMDT)
            nc.vector.tensor_copy(out=ones_r, in_=ones_f)
            eps_sb = const.tile([P, 1], F32)
            nc.vector.memset(eps_sb, EPS)
            rope_sb = const.tile([D, T], F32)
            nc.sync.dma_start(out=rope_sb, in_=rope_t[:, :])
            tri_sb = const.tile([P, P], MMDT)
            nc.sync.dma_start(out=tri_sb, in_=tri[:, :])
            bmask_sb = const.tile([P, 2], F32)
            nc.sync.dma_start(out=bmask_sb, in_=bmask[:, :])

            # ---- DRAM scratch for collectives ----
            a2a1_in = dram.tile([8, 2 * P, TQ], MMDT)
            a2a1_out = dram.tile([8, 2 * P, TQ], MMDT)
            a2a2_in = dram.tile([8, 2 * P, TQ], MMDT)
            a2a2_out = dram.tile([8, 2 * P, TQ], MMDT)

            # ---- persistent SBUF across phase A (q/k/v resident) ----
            qkvp_ctx = tc.tile_pool(name="qkvp", bufs=1)
            qkvp = qkvp_ctx.__enter__()
            qT_sb = qkvp.tile([P, HPC, T], MMDT, tag="qT_sb", bufs=1)
            kT_sb = qkvp.tile([P, HPC, T], MMDT, tag="kT_sb", bufs=1)
            v_sb = qkvp.tile([P, T // P, HPC * D], MMDT, tag="v_sb", bufs=1)

            # ================= Phase A1+A2: rmsnorm1 + QKV (chunked) ============
            with (
                tc.tile_pool(name="p12", bufs=2) as p12,
                tc.tile_pool(name="p12psum", bufs=2, space="PSUM") as pp12,
            ):
                wq_sb = p12.tile([P, CT, P * HPC], MMDT, tag="wq_sb", bufs=1)
                nc.sync.dma_start(out=wq_sb, in_=wq[:, :, :])
                wk_sb = p12.tile([P, CT, P * HPC], MMDT, tag="wk_sb", bufs=1)
                nc.sync.dma_start(out=wk_sb, in_=wk[:, :, :])
                wv_sb = p12.tile([P, CT, P * HPC], MMDT, tag="wv_sb", bufs=1)
                nc.sync.dma_start(out=wv_sb, in_=wv[:, :, :])

                for ch in range(NCH):
                    t0 = ch * TQ
                    xt = p12.tile([P, CT, TQ], MMDT, tag="xt", bufs=2)
                    nc.sync.dma_start(
                        out=xt,
                        in_=x_t[:, t0 : t0 + TQ].rearrange("(ct p) t -> p ct t", p=P),
                    )
                    sq = p12.tile([P, CT, TQ], MMDT, tag="sq", bufs=1)
                    nc.vector.tensor_tensor(
                        out=sq.rearrange("p a b -> p (a b)"),
                        in0=xt.rearrange("p a b -> p (a b)"),
                        in1=xt.rearrange("p a b -> p (a b)"),
                        op=ALU.mult,
                    )
                    ssum = pp12.tile([1, TQ], F32, tag="ssum", bufs=2)
                    for ct in range(CT):
                        nc.tensor.matmul(
                            ssum,
                            ones_r,
                            sq[:, ct, :],
                            start=(ct == 0),
                            stop=(ct == CT - 1),
                        )
                    rstd_row = p12.tile([1, TQ], F32, tag="rstd_row", bufs=2)
                    nc.scalar.activation(
                        rstd_row, ssum, AF.Sqrt, bias=eps_sb[0:1, :], scale=1.0 / C
                    )
                    nc.vector.reciprocal(out=rstd_row, in_=rstd_row)
                    rstd_bc = p12.tile([P, TQ], F32, tag="rstd_bc", bufs=2)
                    nc.gpsimd.partition_broadcast(rstd_bc[:], rstd_row[:])
                    rstd_bch = p12.tile([P, TQ], MMDT, tag="rstd_bch", bufs=2)
                    nc.vector.tensor_copy(out=rstd_bch, in_=rstd_bc)
                    hT = p12.tile([P, CT, TQ], MMDT, tag="hT", bufs=2)
                    for ct in range(CT):
                        nc.vector.tensor_tensor(
                            out=hT[:, ct, :], in0=xt[:, ct, :], in1=rstd_bch, op=ALU.mult
                        )

                    # q^T / k^T with fused rope on eviction (SBUF-resident)
                    HD2 = D // 2
                    for w_sb, dst in ((wq_sb, qT_sb), (wk_sb, kT_sb)):
                        for m in range(HPC):
                            pq = pp12.tile([P, TQ], F32, tag="qk", bufs=3)
                            for ct in range(CT):
                                nc.tensor.matmul(
                                    pq,
                                    w_sb[:, ct, m * P : (m + 1) * P],
                                    hT[:, ct, :],
                                    start=(ct == 0),
                                    stop=(ct == CT - 1),
                                )
                            x1 = pq[0:HD2, :]
                            x2 = pq[HD2:P, :]
                            cosw = rope_sb[0:HD2, t0 : t0 + TQ]
                            sinw = rope_sb[HD2:D, t0 : t0 + TQ]
                            tm1 = p12.tile([HD2, TQ], F32, tag="tm1", bufs=2)
                            tm2 = p12.tile([HD2, TQ], F32, tag="tm2", bufs=2)
                            nc.vector.tensor_tensor(out=tm1, in0=x1, in1=cosw, op=ALU.mult)
                            nc.vector.tensor_tensor(out=tm2, in0=x2, in1=sinw, op=ALU.mult)
                            nc.vector.tensor_tensor(
                                out=dst[0:HD2, m, t0 : t0 + TQ],
                                in0=tm1,
                                in1=tm2,
                                op=ALU.subtract,
                            )
                            nc.vector.tensor_tensor(out=tm1, in0=x1, in1=sinw, op=ALU.mult)
                            nc.vector.tensor_tensor(out=tm2, in0=x2, in1=cosw, op=ALU.mult)
                            nc.vector.tensor_tensor(
                                out=dst[HD2:P, m, t0 : t0 + TQ],
                                in0=tm1,
                                in1=tm2,
                                op=ALU.add,
                            )

                    # v in row layout [t, 4*D], SBUF-resident
                    for rt in range(TQ // P):
                        pv = pp12.tile([P, HPC * D], F32, tag="v", bufs=3)
                        for ct in range(CT):
                            nc.tensor.matmul(
                                pv,
                                hT[:, ct, rt * P : (rt + 1) * P],
                                wv_sb[:, ct, :],
                                start=(ct == 0),
                                stop=(ct == CT - 1),
                            )
                        nc.vector.tensor_copy(
                            out=v_sb[:, ch * (TQ // P) + rt, :], in_=pv
                        )

            # ================= Phase A3: causal attention (+ split A2A) ==========
            with (
                tc.tile_pool(name="att", bufs=2) as att,
                tc.tile_pool(name="attpsum", bufs=2, space="PSUM") as pat,
            ):
                for h in range(HPC):
                    a2a_in = a2a1_in if h < 2 else a2a2_in
                    hrow0 = (h % 2) * P
                    for qc in range(T // TQ):
                        l_ps = pat.tile([1, TQ], F32, tag="l", bufs=1)
                        o_ps = pat.tile([P, TQ], F32, tag="o", bufs=1)
                        es = []
                        # full key-block pairs
                        for kb0 in range(0, 4 * qc, 2):
                            st = pat.tile([P, 2 * TQ], F32, tag="st", bufs=3)
                            for i in range(2):
                                nc.tensor.matmul(
                                    st[:, i * TQ : (i + 1) * TQ],
                                    kT_sb[:, h, (kb0 + i) * P : (kb0 + i + 1) * P],
                                    qT_sb[:, h, qc * TQ : (qc + 1) * TQ],
                                    start=True,
                                    stop=True,
                                )
                            e = att.tile([P, 2 * TQ], MMDT, tag="e", bufs=8)
                            nc.scalar.activation(e, st, AF.Exp, scale=inv_sqrt_d)
                            es.append(("pair", kb0, e))
                        # diagonal blocks (r = 0..3), masked region trimmed
                        for r in range(4):
                            kb = 4 * qc + r
                            q0 = r * P
                            st = pat.tile([P, 2 * TQ], F32, tag="st", bufs=3)
                            nc.tensor.matmul(
                                st[:, q0:TQ],
                                kT_sb[:, h, kb * P : (kb + 1) * P],
                                qT_sb[:, h, qc * TQ + q0 : (qc + 1) * TQ],
                                start=True,
                                stop=True,
                            )
                            e = att.tile([P, 2 * TQ], MMDT, tag="e", bufs=8)
                            nc.scalar.activation(
                                e[:, q0:TQ], st[:, q0:TQ], AF.Exp, scale=inv_sqrt_d
                            )
                            nc.vector.tensor_tensor(
                                out=e[:, q0 : q0 + P],
                                in0=e[:, q0 : q0 + P],
                                in1=tri_sb,
                                op=ALU.mult,
                            )
                            es.append(("diag", kb, e))
                        # denominator + AV accumulation
                        first = True
                        n_items = len(es)
                        for idx, (kind, kb, e) in enumerate(es):
                            last = idx == n_items - 1
                            if kind == "pair":
                                for i in range(2):
                                    nc.tensor.matmul(
                                        l_ps,
                                        ones_r,
                                        e[:, i * TQ : (i + 1) * TQ],
                                        start=first,
                                        stop=False,
                                    )
                                    nc.tensor.matmul(
                                        o_ps,
                                        v_sb[:, kb + i, h * D : (h + 1) * D],
                                        e[:, i * TQ : (i + 1) * TQ],
                                        start=first,
                                        stop=False,
                                    )
                                    first = False
                            else:
                                q0 = (kb - 4 * qc) * P
                                nc.tensor.matmul(
                                    l_ps[:, q0:TQ],
                                    ones_r,
                                    e[:, q0:TQ],
                                    start=first,
                                    stop=last,
                                )
                                nc.tensor.matmul(
                                    o_ps[:, q0:TQ],
                                    v_sb[:, kb, h * D : (h + 1) * D],
                                    e[:, q0:TQ],
                                    start=first,
                                    stop=last,
                                )
                                first = False
                        l_inv = att.tile([1, TQ], F32, tag="l_inv", bufs=2)
                        nc.vector.reciprocal(out=l_inv, in_=l_ps)
                        l_bc = att.tile([P, TQ], F32, tag="l_bc", bufs=2)
                        nc.gpsimd.partition_broadcast(l_bc[:], l_inv[:])
                        oT = att.tile([P, TQ], MMDT, tag="oT", bufs=3)
                        nc.vector.tensor_tensor(out=oT, in0=o_ps, in1=l_bc, op=ALU.mult)
                        # masked writes: own-batch slot gets oT, other-batch zeros
                        oTm0 = att.tile([P, TQ], MMDT, tag="oTm0", bufs=3)
                        nc.vector.tensor_scalar(
                            out=oTm0, in0=oT, scalar1=bmask_sb[:, 0:1],
                            scalar2=None, op0=ALU.mult,
                        )
                        nc.sync.dma_start(out=a2a_in[qc, hrow0 : hrow0 + P, :], in_=oTm0)
                        oTm1 = att.tile([P, TQ], MMDT, tag="oTm1", bufs=3)
                        nc.vector.tensor_scalar(
                            out=oTm1, in0=oT, scalar1=bmask_sb[:, 1:2],
                            scalar2=None, op0=ALU.mult,
                        )
                        nc.sync.dma_start(
                            out=a2a_in[qc + 4, hrow0 : hrow0 + P, :], in_=oTm1
                        )
                    if h == 1:
                        nc.gpsimd.collective_compute(
                            "AllToAll",
                            ALU.bypass,
                            replica_groups=GROUPS,
                            ins=[a2a1_in.opt()],
                            outs=[a2a1_out.opt()],
                        )
                    if h == 3:
                        nc.gpsimd.collective_compute(
                            "AllToAll",
                            ALU.bypass,
                            replica_groups=GROUPS,
                            ins=[a2a2_in.opt()],
                            outs=[a2a2_out.opt()],
                        )
            qkvp_ctx.__exit__(None, None, None)

            # ---- persistent SBUF through phase B ----
            bper_ctx = tc.tile_pool(name="bper", bufs=1)
            bper = bper_ctx.__enter__()
            xmidT = bper.tile([P, CT, TQ], F32, tag="xmidT", bufs=1)
            h2T = bper.tile([P, CT, TQ], MMDT, tag="h2T", bufs=1)

            # ========== Phase B1: proj^T + residual + rmsnorm2 (transposed) ======
            with (
                tc.tile_pool(name="proj", bufs=2) as prj,
                tc.tile_pool(name="projpsum", bufs=2, space="PSUM") as ppj,
            ):
                lp0 = prj.tile([P, 16, TQ], MMDT, tag="lp0", bufs=1)
                lp1 = prj.tile([P, 16, TQ], MMDT, tag="lp1", bufs=1)
                lp0s = prj.tile([P, 8, TQ], MMDT, tag="lp0s", bufs=1)
                lp1s = prj.tile([P, 8, TQ], MMDT, tag="lp1s", bufs=1)
                # cross-batch slots carry zeros; summing s and s+4 keeps own
                # batch.  Loads staggered (blk, blk+8) so sums complete in order.
                for blk in range(8):
                    s_, a_ = blk // 2, blk % 2
                    nc.sync.dma_start(
                        out=lp0[:, blk, :],
                        in_=a2a1_out[s_, a_ * P : (a_ + 1) * P, :],
                    )
                    nc.sync.dma_start(
                        out=lp0[:, blk + 8, :],
                        in_=a2a1_out[s_ + 4, a_ * P : (a_ + 1) * P, :],
                    )
                    nc.vector.tensor_tensor(
                        out=lp0s[:, blk, :],
                        in0=lp0[:, blk, :],
                        in1=lp0[:, blk + 8, :],
                        op=ALU.add,
                    )
                xT_mine = prj.tile([P, CT, TQ], F32, tag="xT_mine", bufs=1)
                nc.sync.dma_start(
                    out=xT_mine, in_=x_tm.rearrange("(ct p) t -> p ct t", p=P)
                )
                # pass 0: heads {0,1} of each sender (a2a1), into xmidT acc
                for ct in range(CT):
                    wpe_sb = prj.tile([P, 8, P], MMDT, tag="wpe_sb", bufs=3)
                    nc.sync.dma_start(out=wpe_sb, in_=wpe_r[0, ct])
                    yps = ppj.tile([P, TQ], F32, tag="y", bufs=4)
                    for blk in range(8):
                        nc.tensor.matmul(
                            yps,
                            wpe_sb[:, blk, :],
                            lp0s[:, blk, :],
                            start=(blk == 0),
                            stop=(blk == 7),
                        )
                    nc.vector.tensor_copy(out=xmidT[:, ct, :], in_=yps)
                # pass 1: heads {2,3} (a2a2) + residual, rmsnorm2 squares per ct
                for blk in range(8):
                    s_, a_ = blk // 2, blk % 2
                    nc.sync.dma_start(
                        out=lp1[:, blk, :],
                        in_=a2a2_out[s_, a_ * P : (a_ + 1) * P, :],
                    )
                    nc.sync.dma_start(
                        out=lp1[:, blk + 8, :],
                        in_=a2a2_out[s_ + 4, a_ * P : (a_ + 1) * P, :],
                    )
                    nc.vector.tensor_tensor(
                        out=lp1s[:, blk, :],
                        in0=lp1[:, blk, :],
                        in1=lp1[:, blk + 8, :],
                        op=ALU.add,
                    )
                sq2 = prj.tile([P, CT, TQ], MMDT, tag="sq2", bufs=1)
                for ct in range(CT):
                    wpe_sb = prj.tile([P, 8, P], MMDT, tag="wpe_sb", bufs=3)
                    nc.sync.dma_start(out=wpe_sb, in_=wpe_r[1, ct])
                    yps = ppj.tile([P, TQ], F32, tag="y", bufs=4)
                    for blk in range(8):
                        nc.tensor.matmul(
                            yps,
                            wpe_sb[:, blk, :],
                            lp1s[:, blk, :],
                            start=(blk == 0),
                            stop=(blk == 7),
                        )
                    t1 = prj.tile([P, TQ], F32, tag="t1", bufs=3)
                    nc.vector.tensor_tensor(
                        out=t1, in0=yps, in1=xmidT[:, ct, :], op=ALU.add
                    )
                    nc.vector.tensor_tensor(
                        out=xmidT[:, ct, :], in0=t1, in1=xT_mine[:, ct, :], op=ALU.add
                    )
                    nc.vector.tensor_tensor(
                        out=sq2[:, ct, :],
                        in0=xmidT[:, ct, :],
                        in1=xmidT[:, ct, :],
                        op=ALU.mult,
                    )
                # rmsnorm2 (transposed): ones-matmul over squares
                ssum2 = ppj.tile([1, TQ], F32, tag="ssum2", bufs=1)
                for ct in range(CT):
                    nc.tensor.matmul(
                        ssum2, ones_r, sq2[:, ct, :], start=(ct == 0), stop=(ct == CT - 1)
                    )
                rstd2 = prj.tile([1, TQ], F32, tag="rstd2", bufs=1)
                nc.scalar.activation(
                    rstd2, ssum2, AF.Sqrt, bias=eps_sb[0:1, :], scale=1.0 / C
                )
                nc.vector.reciprocal(out=rstd2, in_=rstd2)
                rstd2_bc = prj.tile([P, TQ], F32, tag="rstd2_bc", bufs=1)
                nc.gpsimd.partition_broadcast(rstd2_bc[:], rstd2[:])
                for ct in range(CT):
                    nc.vector.tensor_tensor(
                        out=h2T[:, ct, :], in0=xmidT[:, ct, :], in1=rstd2_bc, op=ALU.mult
                    )
            # ================= Phase B2: SwiGLU (transposed w3 pass) =============
            with (
                tc.tile_pool(name="mlp", bufs=2) as mlp,
                tc.tile_pool(name="mlppsum", bufs=2, space="PSUM") as pml,
            ):
                uT = mlp.tile([P, HID_T, TQ], MMDT, tag="uT", bufs=1)
                for ht in range(HID_T):
                    w1_sb = mlp.tile([P, CT, P], MMDT, tag="w1_sb", bufs=3)
                    nc.sync.dma_start(out=w1_sb, in_=w1t[ht])
                    w2_sb = mlp.tile([P, CT, P], MMDT, tag="w2_sb", bufs=3)
                    nc.sync.dma_start(out=w2_sb, in_=w2t[ht])
                    g1 = pml.tile([P, TQ], F32, tag="g1", bufs=2)
                    g2 = pml.tile([P, TQ], F32, tag="g2", bufs=2)
                    for ct in range(CT):
                        nc.tensor.matmul(
                            g1, w1_sb[:, ct, :], h2T[:, ct, :],
                            start=(ct == 0), stop=(ct == CT - 1),
                        )
                    for ct in range(CT):
                        nc.tensor.matmul(
                            g2, w2_sb[:, ct, :], h2T[:, ct, :],
                            start=(ct == 0), stop=(ct == CT - 1),
                        )
                    sil = mlp.tile([P, TQ], F32, tag="sil", bufs=3)
                    nc.scalar.activation(sil, g1, AF.Silu)
                    nc.vector.tensor_tensor(
                        out=uT[:, ht, :], in0=g2, in1=sil, op=ALU.mult
                    )
                # y3^T: stationary w3 blocks, moving uT; accumulate 44 ht per ct
                for ct in range(CT):
                    w3_sb = mlp.tile([P, HID_T, P], MMDT, tag="w3_sb", bufs=2)
                    nc.sync.dma_start(out=w3_sb, in_=w3r[ct])
                    y3 = pml.tile([P, TQ], F32, tag="y3", bufs=2)
                    for ht in range(HID_T):
                        nc.tensor.matmul(
                            y3, w3_sb[:, ht, :], uT[:, ht, :],
                            start=(ht == 0), stop=(ht == HID_T - 1),
                        )
                    ofin = mlp.tile([P, TQ], F32, tag="ofin", bufs=3)
                    nc.vector.tensor_tensor(
                        out=ofin, in0=y3, in1=xmidT[:, ct, :], op=ALU.add
                    )
                    nc.sync.dma_start(out=out[ct * P : (ct + 1) * P, :], in_=ofin)
            bper_ctx.__exit__(None, None, None)

    nc.compile()
    return nc


_NC_CACHE = None


def _get_nc():
    global _NC_CACHE
    if _NC_CACHE is None:
        _NC_CACHE = _build()
    return _NC_CACHE


def _host_inputs(x, w_norm1, w_qkv, w_proj, w_norm2, w1, w2, w3):
    x = np.asarray(x, dtype=np.float32)
    w_qkv = np.asarray(w_qkv, dtype=np.float32)
    w_proj = np.asarray(w_proj, dtype=np.float32)
    w_norm1 = np.asarray(w_norm1, dtype=np.float32)
    w_norm2 = np.asarray(w_norm2, dtype=np.float32)
    w1 = np.asarray(w1, dtype=np.float32)
    w2 = np.asarray(w2, dtype=np.float32)
    w3 = np.asarray(w3, dtype=np.float32)

    half = D // 2
    inv_freq = 1.0 / (ROPE_BASE ** (np.arange(half, dtype=np.float32) / half))
    pos = np.arange(T, dtype=np.float32)
    freqs = pos[:, None] * inv_freq[None, :]
    rope_tab = np.ascontiguousarray(
        np.concatenate([np.cos(freqs).T, np.sin(freqs).T], axis=0).astype(np.float32)
    )

    ql = np.arange(P)[None, :]
    kv = np.arange(P)[:, None]
    tri = (ql >= kv).astype(NP_MMDT)

    # fold w_norm into weight rows (h @ W == (x*rstd) @ (diag(wn) W))
    w_qkv_n = w_qkv * w_norm1[:, None]
    w1_n = w1 * w_norm2[:, None]
    w2_n = w2 * w_norm2[:, None]

    # [HID_T, P, CT*P]: w1t[ht, p, ct*P + d] = w1_n[ct*P + p, ht*P + d]
    w1t = np.ascontiguousarray(
        w1_n.reshape(CT, P, HID_T, P).transpose(2, 1, 0, 3).reshape(HID_T, P, C)
    ).astype(NP_MMDT)
    w2t = np.ascontiguousarray(
        w2_n.reshape(CT, P, HID_T, P).transpose(2, 1, 0, 3).reshape(HID_T, P, C)
    ).astype(NP_MMDT)
    # [CT, P, HID_T*P]: w3r[ct, p, ht*P + d] = w3[ht*P + p, ct*P + d]
    w3r_h = np.ascontiguousarray(
        w3.reshape(HID_T, P, CT, P).transpose(2, 1, 0, 3).reshape(CT, P, HID)
    ).astype(NP_MMDT)

    # [P, CT, cols]: wq[p, ct, d] = w_qkv_n[ct*P + p, col0 + d]
    wqkv_r = np.ascontiguousarray(
        w_qkv_n.reshape(CT, P, 3 * C).transpose(1, 0, 2)
    ).astype(NP_MMDT)

    # wpe: [2(pass hf), CT, P, 8*P], block blk = s*2 + a (sender s in 0..3 of
    # own batch group): w_proj rows of head (4s + 2*hf + a).  Batch-independent
    # (cross-batch neutralization happens via bmask-ed A2A payload).
    wpe_full = np.empty((2, 8, P, C), dtype=np.float32)
    for hf in range(2):
        for s_ in range(4):
            for a in range(2):
                gh = 4 * s_ + hf * 2 + a
                wpe_full[hf, s_ * 2 + a] = w_proj[gh * P : (gh + 1) * P, :]
    wpe_r_h = np.ascontiguousarray(
        wpe_full.reshape(2, 8, P, CT, P).transpose(0, 3, 2, 1, 4).reshape(2, CT, P, 8 * P)
    ).astype(NP_MMDT)

    in_maps = []
    for j in range(8):
        b, hg = j // 4, j % 4
        col0 = hg * HPC * D
        xbT = np.ascontiguousarray(x[b].T)
        bmask_h = np.zeros((P, 2), dtype=np.float32)
        bmask_h[:, b] = 1.0
        in_maps.append(
            {
                "x_t": xbT.astype(NP_MMDT),
                "x_tm": np.ascontiguousarray(xbT[:, hg * TQ : (hg + 1) * TQ]),
                "wq": np.ascontiguousarray(wqkv_r[:, :, col0 : col0 + HPC * D]),
                "wk": np.ascontiguousarray(
                    wqkv_r[:, :, C + col0 : C + col0 + HPC * D]
                ),
                "wv": np.ascontiguousarray(
                    wqkv_r[:, :, 2 * C + col0 : 2 * C + col0 + HPC * D]
                ),
                "wpe_r": wpe_r_h,
                "bmask": bmask_h,
                "w1t": w1t,
                "w2t": w2t,
                "w3r": w3r_h,
                "rope_t": rope_tab,
                "tri": tri,
            }
        )
    return in_maps


def kernel(x, w_norm1, w_qkv, w_proj, w_norm2, w1, w2, w3, _trace=False, _tmpdir=None):
    nc = _get_nc()
    in_maps = _host_inputs(x, w_norm1, w_qkv, w_proj, w_norm2, w1, w2, w3)
    kwargs = {}
    if _trace:
        kwargs = {"trace": True, "tmpdir": _tmpdir}
    res = bass_utils.run_bass_kernel_spmd(
        nc, in_maps, core_ids=list(range(8)), **kwargs
    )
    out = np.empty((2, T, C), dtype=np.float32)
    for j in range(8):
        out[j // 4, (j % 4) * TQ : (j % 4 + 1) * TQ, :] = res.results[j]["out"].T
    kernel._last_exec_time_ns = res.exec_time_ns
    return out


# revision 33
# speedup vs baseline: 1.2633x; 1.0751x over previous
"""Dense transformer block (rmsnorm+causal attention+rope / rmsnorm+SwiGLU) on 8 TRN2 cores.

Sharding:
  core j (j=0..7): batch b = j//4, head-group hg = j%4 (heads 4*hg..4*hg+3).
  Phase A (attention) is head-sharded: each core computes QKV for its 4 heads
  from x^T directly (rmsnorm rstd is folded into the rope tables for q/k and
  applied via a transposed per-row scale for v), then rope -> causal
  attention, with q/k/v kept SBUF-resident.
  Two 8-core AllToAlls (heads {0,1} then {2,3}) reshard to query-sharding.
  Cross-batch payload slots are zeroed via a per-core bmask on the sender, so
  receivers sum slot s and s+4 and contract only 8 real w_proj blocks.
  The second A2A is emitted after the attention pool closes so it overlaps
  the first projection pass.
  Phase B runs fully transposed: proj y^T accumulates [C, TQ] directly,
  rmsnorm2 stats via ones-matmul, SwiGLU with a transposed w3 pass; the
  kernel output is [C, TQ] per core and is transposed on host.

Matmul operands are bf16 (weights and x pre-cast on host, w_norm folded into
weight rows); statistics, softmax denominators, residual stream and PSUM stay
fp32 (residual x arrives separately as fp32 x_tm).
"""

import numpy as np
import ml_dtypes

import concourse.bass as bass
import concourse.mybir as mybir
import concourse.tile as tile
from concourse import bacc
from concourse import bass_utils
from concourse.masks import make_identity

AF = mybir.ActivationFunctionType
ALU = mybir.AluOpType
F32 = mybir.dt.float32
BF16 = mybir.dt.bfloat16
MMDT = BF16
NP_MMDT = ml_dtypes.bfloat16

P = 128
T = 2048
C = 2048
D = 128
H = 16
HPC = 4          # heads per core
HID = 5632
HID_T = HID // P  # 44 hid tiles
TQ = 512         # A2A / output col-block granularity
TQA = 1024       # attention query-chunk
EPS = 1e-6
ROPE_BASE = 10000.0
CT = C // P      # 16 contraction tiles
NCH = 4          # QKV t-chunks of 512


def _build():
    nc = bacc.Bacc(None, target_bir_lowering=False, num_devices=8)

    # ---- kernel I/O ----
    x_t = nc.dram_tensor("x_t", [C, T], MMDT, kind="ExternalInput")
    x_tm = nc.dram_tensor("x_tm", [C, TQ], F32, kind="ExternalInput")
    wq = nc.dram_tensor("wq", [P, CT, HPC * D], MMDT, kind="ExternalInput")
    wk = nc.dram_tensor("wk", [P, CT, HPC * D], MMDT, kind="ExternalInput")
    wv = nc.dram_tensor("wv", [P, CT, HPC * D], MMDT, kind="ExternalInput")
    wpe_r = nc.dram_tensor("wpe_r", [2, CT, P, 8 * P], MMDT, kind="ExternalInput")
    bmask = nc.dram_tensor("bmask", [P, 2], F32, kind="ExternalInput")
    w1t = nc.dram_tensor("w1t", [HID_T, P, CT * P], MMDT, kind="ExternalInput")
    w2t = nc.dram_tensor("w2t", [HID_T, P, CT * P], MMDT, kind="ExternalInput")
    w3r = nc.dram_tensor("w3r", [CT, P, HID_T * P], MMDT, kind="ExternalInput")
    rope_t = nc.dram_tensor("rope_t", [D, T], F32, kind="ExternalInput")
    tri = nc.dram_tensor("tri", [P, P], MMDT, kind="ExternalInput")
    out = nc.dram_tensor("out", [C, TQ], F32, kind="ExternalOutput")

    inv_sqrt_d = 1.0 / float(np.sqrt(D))
    GROUPS = [[0, 1, 2, 3, 4, 5, 6, 7]]
    HD2 = D // 2

    with tile.TileContext(nc) as tc:
        with (
            tc.tile_pool(name="const", bufs=1) as const,
            tc.tile_pool(name="dram", bufs=1, space="DRAM") as dram,
        ):
            # ---- constants ----
            ones_f = const.tile([P, 1], F32)
            nc.vector.memset(ones_f, 1.0)
            ones_r = const.tile([P, 1], MMDT)
            nc.vector.tensor_copy(out=ones_r, in_=ones_f)
            eps_sb = const.tile([P, 1], F32)
            nc.vector.memset(eps_sb, EPS)
            ident_f = const.tile([P, P], F32)
            make_identity(nc, ident_f)
            rope_sb = const.tile([D, T], F32)
            nc.sync.dma_start(out=rope_sb, in_=rope_t[:, :])
            tri_sb = const.tile([P, P], MMDT)
            nc.sync.dma_start(out=tri_sb, in_=tri[:, :])
            bmask_sb = const.tile([P, 2], F32)
            nc.sync.dma_start(out=bmask_sb, in_=bmask[:, :])

            # ---- DRAM scratch for collectives ----
            a2a1_in = dram.tile([8, 2 * P, TQ], MMDT)
            a2a1_out = dram.tile([8, 2 * P, TQ], MMDT)
            a2a2_in = dram.tile([8, 2 * P, TQ], MMDT)
            a2a2_out = dram.tile([8, 2 * P, TQ], MMDT)

            # ---- persistent SBUF across phase A (q/k/v resident) ----
            qkvp_ctx = tc.tile_pool(name="qkvp", bufs=1)
            qkvp = qkvp_ctx.__enter__()
            qT_sb = qkvp.tile([P, HPC, T], MMDT, tag="qT_sb", bufs=1)
            kT_sb = qkvp.tile([P, HPC, T], MMDT, tag="kT_sb", bufs=1)
            v_sb = qkvp.tile([P, T // P, HPC * D], MMDT, tag="v_sb", bufs=1)

            # ================= Phase A1+A2: rmsnorm1 + QKV (chunked) ============
            with (
                tc.tile_pool(name="p12", bufs=2) as p12,
                tc.tile_pool(name="p12psum", bufs=2, space="PSUM") as pp12,
            ):
                first_xt = p12.tile([P, CT, TQ], MMDT, tag="xt", bufs=2)
                nc.sync.dma_start(
                    out=first_xt,
                    in_=x_t[:, 0:TQ].rearrange("(ct p) t -> p ct t", p=P),
                )
                wq_sb = p12.tile([P, CT, P * HPC], MMDT, tag="wq_sb", bufs=1)
                nc.sync.dma_start(out=wq_sb, in_=wq[:, :, :])
                wk_sb = p12.tile([P, CT, P * HPC], MMDT, tag="wk_sb", bufs=1)
                nc.sync.dma_start(out=wk_sb, in_=wk[:, :, :])
                wv_sb = p12.tile([P, CT, P * HPC], MMDT, tag="wv_sb", bufs=1)
                nc.sync.dma_start(out=wv_sb, in_=wv[:, :, :])

                for ch in range(NCH):
                    t0 = ch * TQ
                    if ch == 0:
                        xt = first_xt
                    else:
                        xt = p12.tile([P, CT, TQ], MMDT, tag="xt", bufs=2)
                        nc.sync.dma_start(
                            out=xt,
                            in_=x_t[:, t0 : t0 + TQ].rearrange(
                                "(ct p) t -> p ct t", p=P
                            ),
                        )
                    # rmsnorm stats: squares on ScalarE, partition-sum on PE
                    sq = p12.tile([P, CT, TQ], MMDT, tag="sq", bufs=2)
                    nc.scalar.activation(
                        sq.rearrange("p a b -> p (a b)"),
                        xt.rearrange("p a b -> p (a b)"),
                        AF.Square,
                    )
                    ssum = pp12.tile([1, TQ], F32, tag="ssum", bufs=1)
                    for ct in range(CT):
                        nc.tensor.matmul(
                            ssum,
                            ones_r,
                            sq[:, ct, :],
                            start=(ct == 0),
                            stop=(ct == CT - 1),
                        )
                    srow = p12.tile([1, TQ], F32, tag="srow", bufs=2)
                    nc.scalar.activation(
                        srow, ssum, AF.Sqrt, bias=eps_sb[0:1, :], scale=1.0 / C
                    )
                    rstd_row = p12.tile([1, TQ], F32, tag="rstd_row", bufs=2)
                    nc.vector.reciprocal_approx_fast(out=rstd_row, in_=srow)
                    rstd_bc = p12.tile([P, TQ], F32, tag="rstd_bc", bufs=2)
                    nc.gpsimd.partition_broadcast(rstd_bc[:], rstd_row[:])
                    # rstd folded into rope tables: rows 0:64 cos*rstd, 64:128 sin*rstd
                    cs_r = p12.tile([P, TQ], F32, tag="cs_r", bufs=2)
                    nc.vector.tensor_tensor(
                        out=cs_r, in0=rope_sb[:, t0 : t0 + TQ], in1=rstd_bc,
                        op=ALU.mult,
                    )

                    # q^T / k^T with fused rope(+rstd) on eviction (SBUF-resident)
                    for w_sb, dst in ((wq_sb, qT_sb), (wk_sb, kT_sb)):
                        for m in range(HPC):
                            pq = pp12.tile([P, TQ], F32, tag="qk", bufs=3)
                            for ct in range(CT):
                                nc.tensor.matmul(
                                    pq,
                                    w_sb[:, ct, m * P : (m + 1) * P],
                                    xt[:, ct, :],
                                    start=(ct == 0),
                                    stop=(ct == CT - 1),
                                )
                            x1 = pq[0:HD2, :]
                            x2 = pq[HD2:P, :]
                            cosw = cs_r[0:HD2, :]
                            sinw = cs_r[HD2:P, :]
                            tm1 = p12.tile([HD2, TQ], F32, tag="tm1", bufs=2)
                            tm2 = p12.tile([HD2, TQ], F32, tag="tm2", bufs=2)
                            nc.vector.tensor_tensor(out=tm1, in0=x1, in1=cosw, op=ALU.mult)
                            nc.vector.tensor_tensor(out=tm2, in0=x2, in1=sinw, op=ALU.mult)
                            nc.vector.tensor_tensor(
                                out=dst[0:HD2, m, t0 : t0 + TQ],
                                in0=tm1,
                                in1=tm2,
                                op=ALU.subtract,
                            )
                            nc.vector.tensor_tensor(out=tm1, in0=x1, in1=sinw, op=ALU.mult)
                            nc.vector.tensor_tensor(out=tm2, in0=x2, in1=cosw, op=ALU.mult)
                            nc.vector.tensor_tensor(
                                out=dst[HD2:P, m, t0 : t0 + TQ],
                                in0=tm1,
                                in1=tm2,
                                op=ALU.add,
                            )

                    # v in row layout [t, 4*D]; per-row rstd via PE-transposed col
                    for rt in range(TQ // P):
                        trp = pp12.tile([P, P], F32, tag="trp", bufs=2)
                        nc.tensor.transpose(
                            trp, rstd_bc[:, rt * P : (rt + 1) * P], ident_f
                        )
                        rstd_col = p12.tile([P, 1], F32, tag="rstd_col", bufs=2)
                        nc.vector.tensor_copy(out=rstd_col, in_=trp[:, 0:1])
                        pv = pp12.tile([P, HPC * D], F32, tag="v", bufs=2)
                        for ct in range(CT):
                            nc.tensor.matmul(
                                pv,
                                xt[:, ct, rt * P : (rt + 1) * P],
                                wv_sb[:, ct, :],
                                start=(ct == 0),
                                stop=(ct == CT - 1),
                            )
                        nc.vector.tensor_scalar(
                            out=v_sb[:, ch * (TQ // P) + rt, :],
                            in0=pv,
                            scalar1=rstd_col,
                            scalar2=None,
                            op0=ALU.mult,
                        )

            # ================= Phase A3: causal attention (+ A2A1) ==============
            with (
                tc.tile_pool(name="att", bufs=2) as att,
                tc.tile_pool(name="attpsum", bufs=2, space="PSUM") as pat,
            ):
                for h in range(HPC):
                    a2a_in = a2a1_in if h < 2 else a2a2_in
                    hrow0 = (h % 2) * P
                    for q2 in range(T // TQA):
                        qb = q2 * TQA
                        l_ps = pat.tile([1, TQA], F32, tag="l", bufs=1)
                        o_ps = pat.tile([P, TQA], F32, tag="o", bufs=1)
                        es = []
                        # full key blocks
                        for kb in range(8 * q2):
                            st = pat.tile([P, TQA], F32, tag="st", bufs=2)
                            for i in range(2):
                                nc.tensor.matmul(
                                    st[:, i * TQ : (i + 1) * TQ],
                                    kT_sb[:, h, kb * P : (kb + 1) * P],
                                    qT_sb[:, h, qb + i * TQ : qb + (i + 1) * TQ],
                                    start=True,
                                    stop=True,
                                )
                            e = att.tile([P, TQA], MMDT, tag="e", bufs=18)
                            nc.scalar.activation(e, st, AF.Exp, scale=inv_sqrt_d)
                            es.append((kb, 0, e))
                        # diagonal blocks (r = 0..7), masked region trimmed
                        for r in range(8):
                            kb = 8 * q2 + r
                            q0 = r * P
                            st = pat.tile([P, TQA], F32, tag="st", bufs=2)
                            if q0 < TQ:
                                nc.tensor.matmul(
                                    st[:, q0:TQ],
                                    kT_sb[:, h, kb * P : (kb + 1) * P],
                                    qT_sb[:, h, qb + q0 : qb + TQ],
                                    start=True,
                                    stop=True,
                                )
                                nc.tensor.matmul(
                                    st[:, TQ:TQA],
                                    kT_sb[:, h, kb * P : (kb + 1) * P],
                                    qT_sb[:, h, qb + TQ : qb + TQA],
                                    start=True,
                                    stop=True,
                                )
                            else:
                                nc.tensor.matmul(
                                    st[:, q0:TQA],
                                    kT_sb[:, h, kb * P : (kb + 1) * P],
                                    qT_sb[:, h, qb + q0 : qb + TQA],
                                    start=True,
                                    stop=True,
                                )
                            e = att.tile([P, TQA], MMDT, tag="e", bufs=18)
                            nc.scalar.activation(
                                e[:, q0:TQA], st[:, q0:TQA], AF.Exp, scale=inv_sqrt_d
                            )
                            nc.vector.tensor_tensor(
                                out=e[:, q0 : q0 + P],
                                in0=e[:, q0 : q0 + P],
                                in1=tri_sb,
                                op=ALU.mult,
                            )
                            es.append((kb, q0, e))
                        n_items = len(es)
                        # last writer of bank0 (cols 0:TQ) is the r=3 diag item;
                        # last writer of bank1 is the final (r=7) item
                        b0_last = n_items - 5
                        # denominator pass (stationary ones stays loaded)
                        for idx, (kb, q0, e) in enumerate(es):
                            first = idx == 0
                            if q0 < TQ:
                                nc.tensor.matmul(
                                    l_ps[:, q0:TQ], ones_r, e[:, q0:TQ],
                                    start=first, stop=(idx == b0_last),
                                )
                                nc.tensor.matmul(
                                    l_ps[:, TQ:TQA], ones_r, e[:, TQ:TQA],
                                    start=first, stop=(idx == n_items - 1),
                                )
                            else:
                                nc.tensor.matmul(
                                    l_ps[:, q0:TQA], ones_r, e[:, q0:TQA],
                                    start=first, stop=(idx == n_items - 1),
                                )
                        # AV pass
                        for idx, (kb, q0, e) in enumerate(es):
                            first = idx == 0
                            if q0 < TQ:
                                nc.tensor.matmul(
                                    o_ps[:, q0:TQ],
                                    v_sb[:, kb, h * D : (h + 1) * D],
                                    e[:, q0:TQ],
                                    start=first, stop=(idx == b0_last),
                                )
                                nc.tensor.matmul(
                                    o_ps[:, TQ:TQA],
                                    v_sb[:, kb, h * D : (h + 1) * D],
                                    e[:, TQ:TQA],
                                    start=first, stop=(idx == n_items - 1),
                                )
                            else:
                                nc.tensor.matmul(
                                    o_ps[:, q0:TQA],
                                    v_sb[:, kb, h * D : (h + 1) * D],
                                    e[:, q0:TQA],
                                    start=first, stop=(idx == n_items - 1),
                                )
                        l_inv = att.tile([1, TQA], F32, tag="l_inv", bufs=2)
                        nc.vector.reciprocal_approx_fast(out=l_inv, in_=l_ps)
                        l_bc = att.tile([P, TQA], F32, tag="l_bc", bufs=2)
                        nc.gpsimd.partition_broadcast(l_bc[:], l_inv[:])
                        oT = att.tile([P, TQA], MMDT, tag="oT", bufs=2)
                        nc.vector.tensor_tensor(out=oT, in0=o_ps, in1=l_bc, op=ALU.mult)
                        # masked writes: own-batch slot gets oT, other-batch zeros
                        oTm0 = att.tile([P, TQA], MMDT, tag="oTm0", bufs=2)
                        nc.vector.tensor_scalar(
                            out=oTm0, in0=oT, scalar1=bmask_sb[:, 0:1],
                            scalar2=None, op0=ALU.mult,
                        )
                        oTm1 = att.tile([P, TQA], MMDT, tag="oTm1", bufs=2)
                        nc.vector.tensor_scalar(
                            out=oTm1, in0=oT, scalar1=bmask_sb[:, 1:2],
                            scalar2=None, op0=ALU.mult,
                        )
                        for i in range(2):
                            qc = 2 * q2 + i
                            nc.sync.dma_start(
                                out=a2a_in[qc, hrow0 : hrow0 + P, :],
                                in_=oTm0[:, i * TQ : (i + 1) * TQ],
                            )
                            nc.sync.dma_start(
                                out=a2a_in[qc + 4, hrow0 : hrow0 + P, :],
                                in_=oTm1[:, i * TQ : (i + 1) * TQ],
                            )
                    if h == 1:
                        nc.gpsimd.collective_compute(
                            "AllToAll",
                            ALU.bypass,
                            replica_groups=GROUPS,
                            ins=[a2a1_in.opt()],
                            outs=[a2a1_out.opt()],
                        )
            qkvp_ctx.__exit__(None, None, None)

            # A2A2 emitted outside the attention pool so its completion doesn't
            # gate the pool-close barrier; it overlaps proj pass 0.
            nc.gpsimd.collective_compute(
                "AllToAll",
                ALU.bypass,
                replica_groups=GROUPS,
                ins=[a2a2_in.opt()],
                outs=[a2a2_out.opt()],
            )

            # ---- persistent SBUF through phase B ----
            bper_ctx = tc.tile_pool(name="bper", bufs=1)
            bper = bper_ctx.__enter__()
            xmidT = bper.tile([P, CT, TQ], F32, tag="xmidT", bufs=1)
            h2T = bper.tile([P, CT, TQ], MMDT, tag="h2T", bufs=1)

            # ========== Phase B1: proj^T + residual + rmsnorm2 (transposed) ======
            with (
                tc.tile_pool(name="proj", bufs=2) as prj,
                tc.tile_pool(name="projpsum", bufs=2, space="PSUM") as ppj,
            ):
                lp0 = prj.tile([P, 16, TQ], MMDT, tag="lp0", bufs=1)
                lp1 = prj.tile([P, 16, TQ], MMDT, tag="lp1", bufs=1)
                lp0s = prj.tile([P, 8, TQ], MMDT, tag="lp0s", bufs=1)
                lp1s = prj.tile([P, 8, TQ], MMDT, tag="lp1s", bufs=1)
                # cross-batch slots carry zeros; summing s and s+4 keeps own
                # batch.  Loads staggered (blk, blk+8) so sums complete in order.
                for blk in range(8):
                    s_, a_ = blk // 2, blk % 2
                    nc.sync.dma_start(
                        out=lp0[:, blk, :],
                        in_=a2a1_out[s_, a_ * P : (a_ + 1) * P, :],
                    )
                    nc.sync.dma_start(
                        out=lp0[:, blk + 8, :],
                        in_=a2a1_out[s_ + 4, a_ * P : (a_ + 1) * P, :],
                    )
                    nc.vector.tensor_tensor(
                        out=lp0s[:, blk, :],
                        in0=lp0[:, blk, :],
                        in1=lp0[:, blk + 8, :],
                        op=ALU.add,
                    )
                xT_mine = prj.tile([P, CT, TQ], F32, tag="xT_mine", bufs=1)
                nc.sync.dma_start(
                    out=xT_mine, in_=x_tm.rearrange("(ct p) t -> p ct t", p=P)
                )
                # pass 0: heads {0,1} of each sender (a2a1), into xmidT acc
                for ct in range(CT):
                    wpe_sb = prj.tile([P, 8, P], MMDT, tag="wpe_sb", bufs=3)
                    nc.sync.dma_start(out=wpe_sb, in_=wpe_r[0, ct])
                    yps = ppj.tile([P, TQ], F32, tag="y", bufs=4)
                    for blk in range(8):
                        nc.tensor.matmul(
                            yps,
                            wpe_sb[:, blk, :],
                            lp0s[:, blk, :],
                            start=(blk == 0),
                            stop=(blk == 7),
                        )
                    nc.vector.tensor_copy(out=xmidT[:, ct, :], in_=yps)
                # pass 1: heads {2,3} (a2a2) + residual, rmsnorm2 squares per ct
                for blk in range(8):
                    s_, a_ = blk // 2, blk % 2
                    nc.sync.dma_start(
                        out=lp1[:, blk, :],
                        in_=a2a2_out[s_, a_ * P : (a_ + 1) * P, :],
                    )
                    nc.sync.dma_start(
                        out=lp1[:, blk + 8, :],
                        in_=a2a2_out[s_ + 4, a_ * P : (a_ + 1) * P, :],
                    )
                    nc.vector.tensor_tensor(
                        out=lp1s[:, blk, :],
                        in0=lp1[:, blk, :],
                        in1=lp1[:, blk + 8, :],
                        op=ALU.add,
                    )
                sq2 = prj.tile([P, CT, TQ], MMDT, tag="sq2", bufs=1)
                for ct in range(CT):
                    wpe_sb = prj.tile([P, 8, P], MMDT, tag="wpe_sb", bufs=3)
                    nc.sync.dma_start(out=wpe_sb, in_=wpe_r[1, ct])
                    yps = ppj.tile([P, TQ], F32, tag="y", bufs=4)
                    for blk in range(8):
                        nc.tensor.matmul(
                            yps,
                            wpe_sb[:, blk, :],
                            lp1s[:, blk, :],
                            start=(blk == 0),
                            stop=(blk == 7),
                        )
                    t1 = prj.tile([P, TQ], F32, tag="t1", bufs=3)
                    nc.vector.tensor_tensor(
                        out=t1, in0=yps, in1=xmidT[:, ct, :], op=ALU.add
                    )
                    nc.vector.tensor_tensor(
                        out=xmidT[:, ct, :], in0=t1, in1=xT_mine[:, ct, :], op=ALU.add
                    )
                    nc.scalar.activation(
                        sq2[:, ct, :], xmidT[:, ct, :], AF.Square
                    )
                # rmsnorm2 (transposed): ones-matmul over squares
                ssum2 = ppj.tile([1, TQ], F32, tag="ssum2", bufs=1)
                for ct in range(CT):
                    nc.tensor.matmul(
                        ssum2, ones_r, sq2[:, ct, :], start=(ct == 0), stop=(ct == CT - 1)
                    )
                srow2 = prj.tile([1, TQ], F32, tag="srow2", bufs=1)
                nc.scalar.activation(
                    srow2, ssum2, AF.Sqrt, bias=eps_sb[0:1, :], scale=1.0 / C
                )
                rstd2 = prj.tile([1, TQ], F32, tag="rstd2", bufs=1)
                nc.vector.reciprocal_approx_fast(out=rstd2, in_=srow2)
                rstd2_bc = prj.tile([P, TQ], F32, tag="rstd2_bc", bufs=1)
                nc.gpsimd.partition_broadcast(rstd2_bc[:], rstd2[:])
                for ct in range(CT):
                    nc.vector.tensor_tensor(
                        out=h2T[:, ct, :], in0=xmidT[:, ct, :], in1=rstd2_bc, op=ALU.mult
                    )

            # ================= Phase B2: SwiGLU (transposed w3 pass) =============
            with (
                tc.tile_pool(name="mlp", bufs=2) as mlp,
                tc.tile_pool(name="mlppsum", bufs=2, space="PSUM") as pml,
            ):
                uT = mlp.tile([P, HID_T, TQ], MMDT, tag="uT", bufs=1)
                for ht in range(HID_T):
                    w1_sb = mlp.tile([P, CT, P], MMDT, tag="w1_sb", bufs=3)
                    nc.sync.dma_start(out=w1_sb, in_=w1t[ht])
                    w2_sb = mlp.tile([P, CT, P], MMDT, tag="w2_sb", bufs=3)
                    nc.sync.dma_start(out=w2_sb, in_=w2t[ht])
                    g1 = pml.tile([P, TQ], F32, tag="g1", bufs=2)
                    g2 = pml.tile([P, TQ], F32, tag="g2", bufs=2)
                    for ct in range(CT):
                        nc.tensor.matmul(
                            g1, w1_sb[:, ct, :], h2T[:, ct, :],
                            start=(ct == 0), stop=(ct == CT - 1),
                        )
                    for ct in range(CT):
                        nc.tensor.matmul(
                            g2, w2_sb[:, ct, :], h2T[:, ct, :],
                            start=(ct == 0), stop=(ct == CT - 1),
                        )
                    sil = mlp.tile([P, TQ], F32, tag="sil", bufs=3)
                    nc.scalar.activation(sil, g1, AF.Silu)
                    nc.vector.tensor_tensor(
                        out=uT[:, ht, :], in0=g2, in1=sil, op=ALU.mult
                    )
                # y3^T: stationary w3 blocks, moving uT; accumulate 44 ht per ct
                for ct in range(CT):
                    w3_sb = mlp.tile([P, HID_T, P], MMDT, tag="w3_sb", bufs=2)
                    nc.sync.dma_start(out=w3_sb, in_=w3r[ct])
                    y3 = pml.tile([P, TQ], F32, tag="y3", bufs=2)
                    for ht in range(HID_T):
                        nc.tensor.matmul(
                            y3, w3_sb[:, ht, :], uT[:, ht, :],
                            start=(ht == 0), stop=(ht == HID_T - 1),
                        )
                    ofin = mlp.tile([P, TQ], F32, tag="ofin", bufs=3)
                    nc.vector.tensor_tensor(
                        out=ofin, in0=y3, in1=xmidT[:, ct, :], op=ALU.add
                    )
                    nc.sync.dma_start(out=out[ct * P : (ct + 1) * P, :], in_=ofin)
            bper_ctx.__exit__(None, None, None)

    nc.compile()
    return nc


_NC_CACHE = None


def _get_nc():
    global _NC_CACHE
    if _NC_CACHE is None:
        _NC_CACHE = _build()
    return _NC_CACHE


def _host_inputs(x, w_norm1, w_qkv, w_proj, w_norm2, w1, w2, w3):
    x = np.asarray(x, dtype=np.float32)
    w_qkv = np.asarray(w_qkv, dtype=np.float32)
    w_proj = np.asarray(w_proj, dtype=np.float32)
    w_norm1 = np.asarray(w_norm1, dtype=np.float32)
    w_norm2 = np.asarray(w_norm2, dtype=np.float32)
    w1 = np.asarray(w1, dtype=np.float32)
    w2 = np.asarray(w2, dtype=np.float32)
    w3 = np.asarray(w3, dtype=np.float32)

    half = D // 2
    inv_freq = 1.0 / (ROPE_BASE ** (np.arange(half, dtype=np.float32) / half))
    pos = np.arange(T, dtype=np.float32)
    freqs = pos[:, None] * inv_freq[None, :]
    rope_tab = np.ascontiguousarray(
        np.concatenate([np.cos(freqs).T, np.sin(freqs).T], axis=0).astype(np.float32)
    )

    ql = np.arange(P)[None, :]
    kv = np.arange(P)[:, None]
    tri = (ql >= kv).astype(NP_MMDT)

    # fold w_norm into weight rows (h @ W == (x*rstd) @ (diag(wn) W))
    w_qkv_n = w_qkv * w_norm1[:, None]
    w1_n = w1 * w_norm2[:, None]
    w2_n = w2 * w_norm2[:, None]

    # [HID_T, P, CT*P]: w1t[ht, p, ct*P + d] = w1_n[ct*P + p, ht*P + d]
    w1t = np.ascontiguousarray(
        w1_n.reshape(CT, P, HID_T, P).transpose(2, 1, 0, 3).reshape(HID_T, P, C)
    ).astype(NP_MMDT)
    w2t = np.ascontiguousarray(
        w2_n.reshape(CT, P, HID_T, P).transpose(2, 1, 0, 3).reshape(HID_T, P, C)
    ).astype(NP_MMDT)
    # [CT, P, HID_T*P]: w3r[ct, p, ht*P + d] = w3[ht*P + p, ct*P + d]
    w3r_h = np.ascontiguousarray(
        w3.reshape(HID_T, P, CT, P).transpose(2, 1, 0, 3).reshape(CT, P, HID)
    ).astype(NP_MMDT)

    # [P, CT, cols]: wq[p, ct, d] = w_qkv_n[ct*P + p, col0 + d]
    wqkv_r = np.ascontiguousarray(
        w_qkv_n.reshape(CT, P, 3 * C).transpose(1, 0, 2)
    ).astype(NP_MMDT)

    # wpe: [2(pass hf), CT, P, 8*P], block blk = s*2 + a (sender s in 0..3 of
    # own batch group): w_proj rows of head (4s + 2*hf + a).  Batch-independent
    # (cross-batch neutralization happens via bmask-ed A2A payload).
    wpe_full = np.empty((2, 8, P, C), dtype=np.float32)
    for hf in range(2):
        for s_ in range(4):
            for a in range(2):
                gh = 4 * s_ + hf * 2 + a
                wpe_full[hf, s_ * 2 + a] = w_proj[gh * P : (gh + 1) * P, :]
    wpe_r_h = np.ascontiguousarray(
        wpe_full.reshape(2, 8, P, CT, P).transpose(0, 3, 2, 1, 4).reshape(2, CT, P, 8 * P)
    ).astype(NP_MMDT)

    in_maps = []
    for j in range(8):
        b, hg = j // 4, j % 4
        col0 = hg * HPC * D
        xbT = np.ascontiguousarray(x[b].T)
        bmask_h = np.zeros((P, 2), dtype=np.float32)
        bmask_h[:, b] = 1.0
        in_maps.append(
            {
                "x_t": xbT.astype(NP_MMDT),
                "x_tm": np.ascontiguousarray(xbT[:, hg * TQ : (hg + 1) * TQ]),
                "wq": np.ascontiguousarray(wqkv_r[:, :, col0 : col0 + HPC * D]),
                "wk": np.ascontiguousarray(
                    wqkv_r[:, :, C + col0 : C + col0 + HPC * D]
                ),
                "wv": np.ascontiguousarray(
                    wqkv_r[:, :, 2 * C + col0 : 2 * C + col0 + HPC * D]
                ),
                "wpe_r": wpe_r_h,
                "bmask": bmask_h,
                "w1t": w1t,
                "w2t": w2t,
                "w3r": w3r_h,
                "rope_t": rope_tab,
                "tri": tri,
            }
        )
    return in_maps


def kernel(x, w_norm1, w_qkv, w_proj, w_norm2, w1, w2, w3, _trace=False, _tmpdir=None):
    nc = _get_nc()
    in_maps = _host_inputs(x, w_norm1, w_qkv, w_proj, w_norm2, w1, w2, w3)
    kwargs = {}
    if _trace:
        kwargs = {"trace": True, "tmpdir": _tmpdir}
    res = bass_utils.run_bass_kernel_spmd(
        nc, in_maps, core_ids=list(range(8)), **kwargs
    )
    out = np.empty((2, T, C), dtype=np.float32)
    for j in range(8):
        out[j // 4, (j % 4) * TQ : (j % 4 + 1) * TQ, :] = res.results[j]["out"].T
    kernel._last_exec_time_ns = res.exec_time_ns
    return out
